# revision 1
# baseline (speedup 1.0000x reference)
"""CollapseLoss kernel for Trainium2, 8-way row-sharded.

Reference computation (N=16384 rows, D=128):
    x_n   = row-normalize(feature_clusters)            # F.normalize(dim=1)
    d[i]  = dot(x_n[i+1], x_n[i])        i = 0..N-2
    out   = (d + 1/(N-1))**2

Sharding: 2048 rows per core. Host-side sharding materializes, per core, the
SBUF image xc[128, 17*128]: partition p holds rows 16p..16p+15 of the shard
(blocks 0..15) followed by row 16(p+1) (block 16 — the t=15 partner row,
which for p=127 is the next shard's first row: the halo).  Every device-side
load is then a plain contiguous column-range DMA, and the consecutive-row
dot for (16p+t, 16p+t+1) is a free-dim-shifted product:
    S[:, t] = sum_j AB[:, t*128+j] * AB[:, t*128+128+j]
(the t=15 partner sits right after block 15, so `in1` ranges stay
contiguous across the whole tile).

Work split (default = coarse-grained; knobs in CFG allow the fine-grained
alternative):
  DVE : bulk shifted-product tensor_tensor per group + segmented reduce
        -> S; segmented reduce of the ACT squares -> NE; finals chain
  ACT : one Square pass per group (the last one also covers block 16)

On real TRN2 (measured with K-repeat NEFFs through the PJRT path)
per-instruction fixed costs are ~3-4x the instruction cost model's, so
~18 large instructions beat ~75 fused per-block ones (~28us vs ~44us
steady-state per iteration), even though the cost model prefers the
fine-grained split (12.6us vs 16us modeled).

Tail avoids the inaccurate-rsqrt problem algebraically:
    (S/sqrt(m) + c)^2 == (S + c*sqrt(m))^2 * (1/m)   with m = na*nb
where sqrt(m) only scales the tiny c-term (c ~ 6.1e-5), so ACT's
loose-budget Sqrt table is ample (an integer-magic bitcast sqrt is also
available via act_sqrt=False), and 1/m is the exact DVE reciprocal.  The
ACT activation table is loaded at t~0 via a dummy activation on a const AP
so it never blocks the stream.
"""

import sys
import numpy as np
from contextlib import ExitStack

try:
    import concourse  # noqa: F401
except ImportError:  # grading env without the sitecustomize path
    for _p in ("/opt/trn_rl_repo", "/root/.axon_site/_ro/trn_rl_repo"):
        if _p not in sys.path:
            sys.path.append(_p)

N_ROWS = 16384
D = 128
N_CORES = 8
R = N_ROWS // N_CORES  # 2048 rows per core
P = 128                # partitions
Q = R // P             # 16 row-blocks per partition
C_CONST = 1.0 / (N_ROWS - 1)
SQRT_MAGIC = 0x1FBD1DF5  # bitcast(i>>1 + magic) ~= sqrt, rel err <= 4.5%

# tuning knobs.  The default is the coarse-grained variant: on real TRN2
# hardware (measured via K-repeat NEFFs) per-instruction fixed costs are
# ~3-4x the cost model's, so ~14 big instructions (3 pipelined loads, one
# bulk product+reduce, one Square pass+reduce over all 17 blocks, finals,
# store) beat ~75 fused small ones (~28us vs ~44us steady-state per
# iteration) even though the instruction cost model prefers the
# fine-grained split.
CFG = {
    # input DMA ranges in block units (block 16 = halo/partner), load order
    "load_order": ((0, 6), (6, 12), (12, 17)),
    # compute groups (products/norms emitted per group, in this order)
    "groups": ((0, 16),),
    "pool_groups": (),            # group indices: products via Pool TT
    "bulk_groups": (0,),          # group indices: products via DVE TT+reduce
    "acc_blocks": (),             # norms via ACT Square+accum
    "stt_norm_blocks": (),        # norms via DVE STT (in0=in1=blk)
    "halo_in_pass": True,         # fold block-16 norms into the last pass
    "finals_groups": ((0, 16),),  # [start, end) output block ranges
    "act_sqrt": True,             # ACT table Sqrt vs DVE int-magic sqrt
    "dummy_square": True,         # hoist the ACT table load to t~0
}

_CACHE = {}


def _build_nc(cfg=None, repeat=1):
    import concourse.bacc as bacc
    import concourse.tile as tile
    from concourse import mybir

    cfg = dict(CFG, **(cfg or {}))
    f32 = mybir.dt.float32
    AF = mybir.ActivationFunctionType
    ALU = mybir.AluOpType
    acc_blocks = set(cfg["acc_blocks"])
    stt_norms = set(cfg["stt_norm_blocks"])
    pool_groups = set(cfg["pool_groups"])
    bulk_groups = set(cfg.get("bulk_groups", ()))
    fgroups = cfg["finals_groups"]

    nc = bacc.Bacc(
        "TRN2",
        target_bir_lowering=False,
        debug=False,
        enable_asserts=False,
        num_devices=N_CORES,
    )
    xc = nc.dram_tensor("xc", [P, (Q + 1) * D], f32, kind="ExternalInput").ap()
    out = nc.dram_tensor("out", [R], f32, kind="ExternalOutput").ap()
    out_pq = out.rearrange("(p q) -> p q", p=P)

    with tile.TileContext(nc) as tc:
        with ExitStack() as ctx:
            data = ctx.enter_context(tc.tile_pool(name="data", bufs=1))
            scr = ctx.enter_context(tc.tile_pool(name="scr", bufs=6))
            stat = ctx.enter_context(tc.tile_pool(name="stat", bufs=1))

            # `repeat` exists only for wall-clock benchmarking: the whole
            # pipeline body K times in one NEFF; pool-slot reuse serializes
            # successive repeats like a steady-state stream.
            for _rep in range(repeat):
                AB = data.tile([P, (Q + 1) * D], f32, tag="AB", name=f"AB{_rep}")
                PR = data.tile([P, Q * D], f32)   # pool products / ACT squares
                SQ = data.tile([P, (Q + 1) * D], f32)
                S = stat.tile([P, Q], f32)        # raw consecutive-row dots
                NE = stat.tile([P, Q + 1], f32)   # squared norms incl. block 16

                if cfg["dummy_square"]:
                    # hoist the single ACT table load to t~0; with act_sqrt the
                    # dummy is a Sqrt so the set (sqrt_and_others) covers both
                    dum = scr.tile([P, 1], f32, tag="dum")
                    one = nc.const_aps.aps[(f32, 1.0)]
                    dfn = AF.Sqrt if cfg["act_sqrt"] else AF.Square
                    nc.scalar.activation(out=dum, in_=one[:P], func=dfn)

                for lo, hi in cfg["load_order"]:
                    nc.sync.dma_start(out=AB[:, lo * D:hi * D],
                                      in_=xc[:, lo * D:hi * D])

                # halo norms (block 16): ACT accum or DVE STT, unless folded
                # into the last group's Square pass (halo_in_pass)
                hb = AB[:, Q * D:(Q + 1) * D]
                if cfg.get("halo_in_pass"):
                    pass
                elif Q in stt_norms:
                    sqb = scr.tile([P, D], f32, tag="pr")
                    nc.vector.scalar_tensor_tensor(
                        out=sqb, in0=hb, scalar=1.0, in1=hb,
                        op0=ALU.bypass, op1=ALU.mult, accum_out=NE[:, Q:Q + 1])
                else:
                    sqb = scr.tile([P, D], f32, tag="sq")
                    nc.scalar.activation(out=sqb, in_=hb, func=AF.Square,
                                         accum_out=NE[:, Q:Q + 1])

                fired = set()
                done_blocks = set()
                for gidx, (ba, bb) in enumerate(cfg["groups"]):
                    lo, hi = ba * D, bb * D
                    # products (in1 spans one block past, contiguous incl. halo)
                    if gidx in pool_groups or gidx in bulk_groups:
                        peng = nc.gpsimd if gidx in pool_groups else nc.vector
                        peng.tensor_tensor(out=PR[:, lo:hi],
                                           in0=AB[:, lo:hi],
                                           in1=AB[:, lo + D:hi + D],
                                           op=ALU.mult)
                        nc.vector.tensor_reduce(
                            S[:, ba:bb],
                            PR[:, lo:hi].rearrange("p (q d) -> p q d", q=bb - ba),
                            axis=mybir.AxisListType.X, op=ALU.add)
                    else:
                        for t in range(ba, bb):
                            blk = AB[:, t * D:(t + 1) * D]
                            nxt = AB[:, (t + 1) * D:(t + 2) * D]
                            pr = scr.tile([P, D], f32, tag="pr", name=f"pr{t}")
                            nc.vector.scalar_tensor_tensor(
                                out=pr, in0=blk, scalar=1.0, in1=nxt,
                                op0=ALU.bypass, op1=ALU.mult,
                                accum_out=S[:, t:t + 1])

                    # norms
                    for t in [t for t in range(ba, bb) if t in stt_norms]:
                        blk = AB[:, t * D:(t + 1) * D]
                        sqt = scr.tile([P, D], f32, tag="pr", name=f"sqs{t}")
                        nc.vector.scalar_tensor_tensor(
                            out=sqt, in0=blk, scalar=1.0, in1=blk,
                            op0=ALU.bypass, op1=ALU.mult,
                            accum_out=NE[:, t:t + 1])
                    for t in [t for t in range(ba, bb) if t in acc_blocks]:
                        blk = AB[:, t * D:(t + 1) * D]
                        sqt = scr.tile([P, D], f32, tag="sq", name=f"sqa{t}")
                        nc.scalar.activation(out=sqt, in_=blk, func=AF.Square,
                                             accum_out=NE[:, t:t + 1])
                    run = []
                    ptl = [t for t in range(ba, bb)
                           if t not in acc_blocks and t not in stt_norms]
                    if cfg.get("halo_in_pass") and bb == Q:
                        ptl.append(Q)  # fold block 16 into the final run
                    for t in ptl + [None]:
                        if run and (t is None or t != run[-1] + 1):
                            a, b = run[0], run[-1] + 1
                            if cfg.get("sq_bulk_dve"):
                                nc.vector.tensor_tensor(
                                    out=SQ[:, a * D:b * D],
                                    in0=AB[:, a * D:b * D],
                                    in1=AB[:, a * D:b * D], op=ALU.mult)
                            else:
                                nc.scalar.activation(out=SQ[:, a * D:b * D],
                                                     in_=AB[:, a * D:b * D],
                                                     func=AF.Square)
                            nc.vector.tensor_reduce(
                                NE[:, a:b],
                                SQ[:, a * D:b * D].rearrange(
                                    "p (q d) -> p q d", q=b - a),
                                axis=mybir.AxisListType.X, op=ALU.add)
                            run = []
                        if t is not None:
                            run.append(t)

                    # finals for any output group now fully determined
                    done_blocks.update(range(ba, bb))
                    for gi, (ga, gb) in enumerate(fgroups):
                        need = gb + 1 if gb < Q else Q
                        if gi not in fired and done_blocks >= set(range(ga, need)):
                            fired.add(gi)
                            _emit_finals(nc, stat, mybir, S, NE, out_pq,
                                         ga, gb, gi, cfg)

    nc.compile()
    return nc


def _emit_finals(nc, stat, mybir, S, NE, out_pq, ga, gb, gi, cfg):
    """out[:, ga:gb] = (S + c*sqrt(m))^2 / m for block range [ga, gb)."""
    ALU = mybir.AluOpType
    f32 = mybir.dt.float32
    i32 = mybir.dt.int32
    AF = mybir.ActivationFunctionType
    w_ = gb - ga
    m = stat.tile([P, w_], f32, name=f"m{gi}")
    nc.vector.tensor_tensor(out=m, in0=NE[:, ga:gb], in1=NE[:, ga + 1:gb + 1],
                            op=ALU.mult)
    w = stat.tile([P, w_], f32, name=f"w{gi}")
    nc.vector.reciprocal(w, m)   # off the sqrt chain; joins at the end
    s0 = stat.tile([P, w_], f32, name=f"s0{gi}")
    if cfg["act_sqrt"]:
        nc.scalar.activation(out=s0, in_=m, func=AF.Sqrt)
    else:
        sh = stat.tile([P, w_], f32, name=f"sh{gi}")
        nc.vector.tensor_scalar(sh.bitcast(i32), m.bitcast(i32), 1, None,
                                ALU.logical_shift_right)
        nc.vector.tensor_scalar(s0.bitcast(i32), sh.bitcast(i32), SQRT_MAGIC,
                                None, ALU.add)
    u = stat.tile([P, w_], f32, name=f"u{gi}")
    nc.vector.scalar_tensor_tensor(out=u, in0=s0, scalar=C_CONST,
                                   in1=S[:, ga:gb], op0=ALU.mult, op1=ALU.add)
    v = stat.tile([P, w_], f32, name=f"v{gi}")
    nc.vector.tensor_tensor(out=v, in0=u, in1=u, op=ALU.mult)
    o = stat.tile([P, w_], f32, name=f"o{gi}")
    nc.vector.tensor_tensor(out=o, in0=v, in1=w, op=ALU.mult)
    nc.sync.dma_start(out=out_pq[:, ga:gb], in_=o)


def _get_nc():
    if "nc" not in _CACHE:
        _CACHE["nc"] = _build_nc()
    return _CACHE["nc"]


def make_in_maps(x: np.ndarray) -> list[dict[str, np.ndarray]]:
    """Host-side sharding: build each core's SBUF image xc[128, 2176]."""
    x = np.ascontiguousarray(np.asarray(x, dtype=np.float32))
    # pad one row (the out-of-range halo of the last core) with ones
    xp = np.concatenate([x, np.ones((1, D), dtype=np.float32)], axis=0)
    in_maps = []
    for c in range(N_CORES):
        sh = xp[c * R:c * R + R].reshape(P, Q * D)        # blocks 0..15
        halo = xp[c * R + 16 * np.arange(1, P + 1)]       # block 16
        xc = np.concatenate([sh, halo.reshape(P, D)], axis=1)
        in_maps.append({"xc": np.ascontiguousarray(xc)})
    return in_maps


def kernel(feature_clusters: np.ndarray) -> np.ndarray:
    from concourse.bass_utils import run_bass_kernel_spmd

    nc = _get_nc()
    in_maps = make_in_maps(feature_clusters)
    res = run_bass_kernel_spmd(nc, in_maps, list(range(N_CORES))).results
    full = np.concatenate([res[c]["out"] for c in range(N_CORES)])
    return full[:N_ROWS - 1].astype(np.float32)



# revision 2
# speedup vs baseline: 1.2434x; 1.2434x over previous
"""CollapseLoss kernel for Trainium2, 8-way row-sharded, fp16 datapath.

Reference computation (N=16384 rows, D=128):
    x_n   = row-normalize(feature_clusters)            # F.normalize(dim=1)
    d[i]  = dot(x_n[i+1], x_n[i])        i = 0..N-2
    out   = (d + 1/(N-1))**2

Sharding: 2048 rows per core.  Host-side sharding materializes, per core, the
SBUF image xh[128, 17*128] IN FP16: partition p holds rows 16p..16p+15 of the
shard (blocks 0..15) followed by row 16(p+1) (block 16 — the t=15 partner
row; for p=127 it is the next shard's first row: the halo).  fp16 halves HBM
traffic (the memory roofline) and unlocks the DVE 2x_1p perf mode.

Raw (unnormalized) dots S and squared norms NE are computed in fp16 and
reduced per 128-row block; normalization happens algebraically in the finals:
    out = (S + c*sqrt(m))^2 / m,   m = NE[t] * NE[t+1]
which avoids the inaccurate rsqrt table (sqrt only scales the tiny c term).

Engine split:
  DVE : bulk shifted products (TT, 2x fp16), halving fold chains + short
        TensorReduce for the per-block sums (TensorReduce has no fp16 fast
        path, so fold 128->16 with 2x TT adds first), finals chain.
  ACT : bulk Square passes (products of squares feed the NE folds), final
        sqrt; activation table hoisted to t~0 via a dummy op.
fp16 numerics: |x| < 5.2, |S| < ~60, NE ~ 128+-16 all well inside fp16
range; end-to-end L2 rel err ~4e-3 vs the 2e-2 gate.
"""

import sys
import numpy as np
from contextlib import ExitStack

try:
    import concourse  # noqa: F401
except ImportError:  # grading env without the sitecustomize path
    for _p in ("/opt/trn_rl_repo", "/root/.axon_site/_ro/trn_rl_repo"):
        if _p not in sys.path:
            sys.path.append(_p)

N_ROWS = 16384
D = 128
N_CORES = 8
R = N_ROWS // N_CORES  # 2048 rows per core
P = 128                # partitions
Q = R // P             # 16 row-blocks per partition
C_CONST = 1.0 / (N_ROWS - 1)

CFG = {
    # input DMA chunks in block units (block 16 = halo/partner row block)
    "load_order": ((0, 6), (6, 12), (12, 17)),
    # compute ranges (products/squares emitted per range, finals per group)
    "fold_to": 16,      # fold block width down to this, then TensorReduce
    "store_mode": "hwdge",   # hwdge | kv
    "repeat": 1,
}

_CACHE = {}


def _ranges_from_loads(load_order):
    """Derive product ranges / square ranges / finals groups from chunks.

    Products for blocks [a,b) read AB blocks a..b, so a product range can
    only fire once the chunk holding block b has landed.
    """
    bounds = [lo for lo, _ in load_order] + [load_order[-1][1]]
    prod, sq, fin = [], [], []
    pstart = 0
    for i in range(1, len(bounds)):
        end = bounds[i]
        sq.append((bounds[i - 1], end))
        pend = min(end - 1, Q) if i < len(bounds) - 1 else Q
        if pend > pstart:
            prod.append((pstart, pend))
            fin.append((pstart, pend))
            pstart = pend
    return prod, sq, fin


def _build_nc(cfg=None):
    import concourse.bacc as bacc
    import concourse.tile as tile
    from concourse import mybir

    cfg = dict(CFG, **(cfg or {}))
    f32 = mybir.dt.float32
    f16 = mybir.dt.float16
    AF = mybir.ActivationFunctionType
    ALU = mybir.AluOpType
    X = mybir.AxisListType.X
    fold_to = cfg["fold_to"]

    nc = bacc.Bacc(
        "TRN2",
        target_bir_lowering=False,
        debug=False,
        enable_asserts=False,
        num_devices=N_CORES,
    )
    xh = nc.dram_tensor("xh", [P, (Q + 1) * D], f16, kind="ExternalInput").ap()
    out = nc.dram_tensor("out", [R], f32, kind="ExternalOutput").ap()
    out_pq = out.rearrange("(p q) -> p q", p=P)

    prod_ranges, sq_ranges, fin_groups = _ranges_from_loads(cfg["load_order"])

    with tile.TileContext(nc) as tc:
        with ExitStack() as ctx:
            data = ctx.enter_context(tc.tile_pool(name="data", bufs=1))
            scr = ctx.enter_context(tc.tile_pool(name="scr", bufs=2))
            stat = ctx.enter_context(tc.tile_pool(name="stat", bufs=1))

            for _rep in range(cfg["repeat"]):
                AB = data.tile([P, (Q + 1) * D], f16, tag="AB", name=f"AB{_rep}")
                PR = data.tile([P, Q * D], f16)       # shifted products
                SQ = data.tile([P, (Q + 1) * D], f16)  # squares
                # fold scratch, product side and square side
                FP1 = data.tile([P, Q * 64], f16)
                FP2 = data.tile([P, Q * 32], f16)
                FP3 = data.tile([P, Q * 16], f16)
                FQ1 = data.tile([P, (Q + 1) * 64], f16)
                FQ2 = data.tile([P, (Q + 1) * 32], f16)
                FQ3 = data.tile([P, (Q + 1) * 16], f16)
                S = stat.tile([P, Q], f32)            # raw consecutive-row dots
                NE = stat.tile([P, Q + 1], f32)       # squared norms incl blk 16
                OUTB = stat.tile([P, Q], f32)         # final outputs

                # hoist the single ACT table load (Sqrt set) to t~0
                dum = scr.tile([P, 1], f32, tag="dum")
                one = nc.const_aps.aps[(f32, 1.0)]
                nc.scalar.activation(out=dum, in_=one[:P], func=AF.Sqrt)

                for lo, hi in cfg["load_order"]:
                    nc.sync.dma_start(out=AB[:, lo * D:hi * D],
                                      in_=xh[:, lo * D:hi * D])

                AB3 = AB.rearrange("p (q d) -> p q d", q=Q + 1)
                PR3 = PR.rearrange("p (q d) -> p q d", q=Q)
                SQ3 = SQ.rearrange("p (q d) -> p q d", q=Q + 1)

                def fold_chain(src3, a, b, w1, w2, w3, sink, eng=nc.vector):
                    """src3[:, a:b, 0:128] -> per-block sums into sink[:, a:b]."""
                    n = b - a
                    v1 = w1.rearrange("p (q d) -> p q d", q=w1.shape[1] // 64)
                    v2 = w2.rearrange("p (q d) -> p q d", q=w2.shape[1] // 32)
                    v3 = w3.rearrange("p (q d) -> p q d", q=w3.shape[1] // 16)
                    eng.tensor_tensor(out=v1[:, a:b], in0=src3[:, a:b, 0:64],
                                      in1=src3[:, a:b, 64:128], op=ALU.add)
                    cur, width = v1, 64
                    for nxt, nw in ((v2, 32), (v3, 16)):
                        if width <= fold_to:
                            break
                        eng.tensor_tensor(out=nxt[:, a:b],
                                          in0=cur[:, a:b, 0:nw],
                                          in1=cur[:, a:b, nw:2 * nw], op=ALU.add)
                        cur, width = nxt, nw
                    eng.tensor_reduce(sink[:, a:b], cur[:, a:b], axis=X, op=ALU.add)

                done_sq = 0
                fired = set()
                for ri, (pa, pb) in enumerate(prod_ranges):
                    # squares on ACT for the blocks this chunk delivered
                    qa, qb = sq_ranges[ri]
                    nc.scalar.activation(out=SQ[:, qa * D:qb * D],
                                         in_=AB[:, qa * D:qb * D], func=AF.Square)
                    done_sq = qb
                    # shifted products on DVE
                    nc.vector.tensor_tensor(out=PR[:, pa * D:pb * D],
                                            in0=AB[:, pa * D:pb * D],
                                            in1=AB[:, pa * D + D:pb * D + D],
                                            op=ALU.mult)
                    fold_chain(PR3, pa, pb, FP1, FP2, FP3, S)
                    fold_chain(SQ3, qa, qb, FQ1, FQ2, FQ3, NE)

                    for gi, (ga, gb) in enumerate(fin_groups):
                        need_ne = gb + 1
                        if gi not in fired and done_sq >= need_ne and pb >= gb:
                            fired.add(gi)
                            _emit_finals(nc, stat, mybir, S, NE, OUTB, ga, gb, gi)

                nc.sync.dma_start(out=out_pq, in_=OUTB)

    nc.compile()
    return nc


def _emit_finals(nc, stat, mybir, S, NE, OUTB, ga, gb, gi):
    """OUTB[:, ga:gb] = (S + c*sqrt(m))^2 / m for block range [ga, gb)."""
    ALU = mybir.AluOpType
    f32 = mybir.dt.float32
    AF = mybir.ActivationFunctionType
    w_ = gb - ga
    m = stat.tile([P, w_], f32, name=f"m{gi}")
    nc.vector.tensor_tensor(out=m, in0=NE[:, ga:gb], in1=NE[:, ga + 1:gb + 1],
                            op=ALU.mult)
    w = stat.tile([P, w_], f32, name=f"w{gi}")
    nc.vector.reciprocal(w, m)   # off the sqrt chain; joins at the end
    s0 = stat.tile([P, w_], f32, name=f"s0{gi}")
    nc.scalar.activation(out=s0, in_=m, func=AF.Sqrt)
    u = stat.tile([P, w_], f32, name=f"u{gi}")
    nc.vector.scalar_tensor_tensor(out=u, in0=s0, scalar=C_CONST,
                                   in1=S[:, ga:gb], op0=ALU.mult, op1=ALU.add)
    v = stat.tile([P, w_], f32, name=f"v{gi}")
    nc.vector.tensor_tensor(out=v, in0=u, in1=u, op=ALU.mult)
    nc.vector.tensor_tensor(out=OUTB[:, ga:gb], in0=v, in1=w, op=ALU.mult)


def _get_nc():
    if "nc" not in _CACHE:
        _CACHE["nc"] = _build_nc()
    return _CACHE["nc"]


def make_in_maps(x: np.ndarray) -> list[dict[str, np.ndarray]]:
    """Host-side sharding: build each core's SBUF image xh[128, 2176] fp16."""
    x = np.asarray(x, dtype=np.float32).astype(np.float16)
    # pad one row (the out-of-range halo of the last core) with ones
    xp = np.concatenate([x, np.ones((1, D), dtype=np.float16)], axis=0)
    in_maps = []
    for c in range(N_CORES):
        sh = xp[c * R:c * R + R].reshape(P, Q * D)        # blocks 0..15
        halo = xp[c * R + 16 * np.arange(1, P + 1)]       # block 16
        xh = np.concatenate([sh, halo.reshape(P, D)], axis=1)
        in_maps.append({"xh": np.ascontiguousarray(xh)})
    return in_maps


def kernel(feature_clusters: np.ndarray) -> np.ndarray:
    from concourse.bass_utils import run_bass_kernel_spmd

    nc = _get_nc()
    in_maps = make_in_maps(feature_clusters)
    res = run_bass_kernel_spmd(nc, in_maps, list(range(N_CORES))).results
    full = np.concatenate([res[c]["out"] for c in range(N_CORES)])
    return full[:N_ROWS - 1].astype(np.float32)


# revision 11
# speedup vs baseline: 1.6286x; 1.3099x over previous
"""CollapseLoss kernel for Trainium2, 8-way row-sharded, fp16 datapath.

Reference computation (N=16384 rows, D=128):
    x_n   = row-normalize(feature_clusters)            # F.normalize(dim=1)
    d[i]  = dot(x_n[i+1], x_n[i])        i = 0..N-2
    out   = (d + 1/(N-1))**2

Sharding: 2048 rows per core.  Host-side sharding materializes, per core, the
SBUF image xh[128, 17*128] IN FP16: partition p holds rows 16p..16p+15 of the
shard (blocks 0..15) followed by row 16(p+1) (block 16 — the t=15 partner
row; for p=127 it is the next shard's first row: the halo).  fp16 halves the
HBM roofline and unlocks the DVE 2x_1p perf mode.

Raw (unnormalized) dots S and squared norms NE are computed in fp16; the
normalization happens algebraically in the f32 finals:
    out = (S + c*sqrt(m))^2 / m,   m = NE[t] * NE[t+1]
(sqrt only scales the tiny c term, so the loose ACT Sqrt table is ample).

Structure / engine split:
  - First input chunk is a SWDGE dma_gather prepared at t~0 on GPSIMD and
    fired by trigger_dma: the transfer starts ~800ns before the HWDGE path
    could.  Remaining chunks ride HWDGE (desc-gen pipelines under chunk 1's
    transfer).  Identity gather indices come from a GPSIMD iota.
  - Products (DVE TT) and squares (ACT Square) write an interleaved
    PRSQ[p, t, {prod,sq}, 128] buffer so ONE halving fold chain
    (TT adds at fp16 2x) + one short TensorReduce yields the interleaved
    [S, NE] per-block sums — TensorReduce has no fp16 fast path, so folding
    128->16 first is ~2x cheaper than a straight reduce.
  - Block 16's norm (the halo) goes through an ACT Square+accum.
  - The output store is a SWDGE dma_scatter_add prepared early and fired by
    trigger_dma after the finals: tail cost is ~60ns dispatch + transfer
    + sem instead of the HWDGE's 625+650 desc-gen/DGE latency.  The DRAM
    out buffer ([128, 64] padded rows, host slices [:, :16]) is zeroed by an
    early inline GPSIMD store of the memset OUTB tile (scatter ADDs).
"""

import sys
import numpy as np
from contextlib import ExitStack

try:
    import concourse  # noqa: F401
except ImportError:  # grading env without the sitecustomize path
    for _p in ("/opt/trn_rl_repo", "/root/.axon_site/_ro/trn_rl_repo"):
        if _p not in sys.path:
            sys.path.append(_p)

N_ROWS = 16384
D = 128
N_CORES = 8
R = N_ROWS // N_CORES  # 2048 rows per core
P = 128                # partitions
Q = R // P             # 16 row-blocks per partition
C_CONST = 1.0 / (N_ROWS - 1)
OUTW = 64              # padded out row width (scatter elem must be 256B)

CFG = {
    # (lo, hi, mode): input chunks in block units; block 16 = halo blk.
    # NOTE: "gather" (SWDGE dma_gather prep+trigger) starts the first
    # transfer ~800ns earlier in the model but produced flaky data/crashes
    # on the real axon cores, so loads stay on the HWDGE path.
    "chunks": ((0, 6, "hwdge"), (6, 12, "hwdge"), (12, 17, "hwdge")),
    "finals_groups": ((0, 11), (11, 16)),
    "fold_to": 16,
    "store_mode": "kv",   # kv | hwdge
    "repeat": 1,
}

_CACHE = {}


def _plan(chunks):
    """Product ranges and paired-fold ranges implied by the chunk bounds.

    Products for blocks [a,b) read AB blocks a..b; a fold pair t needs both
    the product t and the square t (squares land with their chunk).
    """
    prod, pstart = [], 0
    for i, (lo, hi, _m) in enumerate(chunks):
        last = i == len(chunks) - 1
        pend = Q if last else min(hi - 1, Q)
        if pend > pstart:
            prod.append((pstart, pend))
            pstart = pend
    return prod


def _build_nc(cfg=None):
    import concourse.bacc as bacc
    import concourse.tile as tile
    from concourse import mybir, library_config

    cfg = dict(CFG, **(cfg or {}))
    f32 = mybir.dt.float32
    f16 = mybir.dt.float16
    i16 = mybir.dt.int16
    i32 = mybir.dt.int32
    AF = mybir.ActivationFunctionType
    ALU = mybir.AluOpType
    X = mybir.AxisListType.X
    fold_to = cfg["fold_to"]
    chunks = cfg["chunks"]
    use_kv = cfg["store_mode"] == "kv"
    any_gather = any(m == "gather" for _, _, m in chunks)

    nc = bacc.Bacc(
        "TRN2",
        target_bir_lowering=False,
        debug=False,
        enable_asserts=False,
        num_devices=N_CORES,
    )
    xh = nc.dram_tensor("xh", [P, (Q + 1) * D], f16, kind="ExternalInput").ap()
    out = nc.dram_tensor("out", [P, OUTW], f32, kind="ExternalOutput").ap()

    prod_ranges = _plan(chunks)

    with tile.TileContext(nc) as tc:
        with ExitStack() as ctx:
            data = ctx.enter_context(tc.tile_pool(name="data", bufs=1))
            scr = ctx.enter_context(tc.tile_pool(name="scr", bufs=2))
            stat = ctx.enter_context(tc.tile_pool(name="stat", bufs=1))

            for _rep in range(cfg["repeat"]):
                AB = data.tile([P, (Q + 1) * D], f16, name=f"AB{_rep}")
                PRSQ = data.tile([P, 2 * Q * D], f16)   # [t, {prod, sq}, 128]
                F1 = data.tile([P, 2 * Q * 64], f16)
                F2 = data.tile([P, 2 * Q * 32], f16)
                F3 = data.tile([P, 2 * Q * 16], f16)
                SN = stat.tile([P, 2 * Q], f32)         # interleaved S/NE
                NE16 = stat.tile([P, 1], f32)           # halo block norm
                OUTB = stat.tile([P, OUTW], f32)

                # ---- GPSIMD stream: library, identity idxs, preps/triggers
                if any_gather or use_kv:
                    nc.gpsimd.load_library(library_config.attnmlp)
                if any_gather:
                    idxs = scr.tile([16, 8], i16, name=f"idx{_rep}")
                    # slot i of the SWDGE ring reads idxs[i%16, i//16]; we
                    # want slot i -> row i (identity).
                    nc.gpsimd.iota(idxs, pattern=[[16, 8]], base=0,
                                   channel_multiplier=1)

                # hoist the single ACT table load (Sqrt set) to t~0
                dum = scr.tile([P, 1], f32, name=f"dum{_rep}")
                one = nc.const_aps.aps[(f32, 1.0)]
                nc.scalar.activation(out=dum, in_=one[:P], func=AF.Sqrt)

                # ---- input loads
                for ci, (lo, hi, mode) in enumerate(chunks):
                    w = (hi - lo) * D
                    if mode == "gather":
                        sem = nc.alloc_semaphore(f"gat{_rep}_{ci}")
                        dst = AB[:, lo * D:hi * D].rearrange(
                            "p (s e) -> p s e", s=1)
                        nc.gpsimd.dma_gather(
                            dst, xh[:, lo * D:hi * D], idxs, P, P, w,
                            elem_step=(Q + 1) * D,
                            prepare_only=True, sem=sem)
                        nc.gpsimd.trigger_dma(count=None)
                    else:
                        nc.sync.dma_start(out=AB[:, lo * D:hi * D],
                                          in_=xh[:, lo * D:hi * D])

                # ---- store prep (early; OUTB read deferred to the trigger)
                if use_kv:
                    ctxz = scr.tile([P, 1], i32, name=f"ctx{_rep}")
                    nc.gpsimd.memset(ctxz, 0)
                    ssem = nc.alloc_semaphore(f"kv{_rep}")
                    out4 = out.rearrange("p (b o q) -> b p o q", b=1, o=1)
                    in4 = OUTB[:, 0:Q].rearrange("p (o b q) -> p o b q",
                                                 o=1, b=1)
                    nc.gpsimd.kv_writeback(out4, in4, ctxz,
                                           prepare_only=True, sem=ssem)

                AB3 = AB.rearrange("p (q d) -> p q d", q=Q + 1)
                PRSQt = PRSQ.rearrange("p (t x) -> p t x", t=Q)   # x = 2*128
                PRSQu = PRSQ.rearrange("p (u d) -> p u d", u=2 * Q)
                V1 = F1.rearrange("p (u d) -> p u d", u=2 * Q)
                V2 = F2.rearrange("p (u d) -> p u d", u=2 * Q)
                V3 = F3.rearrange("p (u d) -> p u d", u=2 * Q)

                def fold_pairs(fa, fb):
                    """Fold PRSQ pair-blocks [fa,fb) down to SN[:, 2fa:2fb]."""
                    ua, ub = 2 * fa, 2 * fb
                    nc.vector.tensor_tensor(
                        out=V1[:, ua:ub], in0=PRSQu[:, ua:ub, 0:64],
                        in1=PRSQu[:, ua:ub, 64:128], op=ALU.add)
                    cur, width = V1, 64
                    for nxt, nw in ((V2, 32), (V3, 16)):
                        if width <= fold_to:
                            break
                        nc.vector.tensor_tensor(
                            out=nxt[:, ua:ub], in0=cur[:, ua:ub, 0:nw],
                            in1=cur[:, ua:ub, nw:2 * nw], op=ALU.add)
                        cur, width = nxt, nw
                    nc.vector.tensor_reduce(SN[:, ua:ub], cur[:, ua:ub],
                                            axis=X, op=ALU.add)

                fin_groups = cfg["finals_groups"]
                fired = set()
                fold_done = 0
                sq_done = 0
                for ri, (pa, pb) in enumerate(prod_ranges):
                    lo, hi, _m = chunks[ri]
                    # squares on ACT (prod lane t<16 only; halo via accum)
                    qa, qb = sq_done, min(hi, Q)
                    if qb > qa:
                        nc.scalar.activation(
                            out=PRSQt[:, qa:qb, D:2 * D],
                            in_=AB3[:, qa:qb, :], func=AF.Square)
                        sq_done = qb
                    if hi == Q + 1:
                        sqh = scr.tile([P, D], f16, name=f"sqh{_rep}")
                        nc.scalar.activation(out=sqh, in_=AB3[:, Q, :],
                                             func=AF.Square, accum_out=NE16)
                    # shifted products on DVE
                    nc.vector.tensor_tensor(
                        out=PRSQt[:, pa:pb, 0:D],
                        in0=AB3[:, pa:pb, :],
                        in1=AB[:, pa * D + D:pb * D + D].rearrange(
                            "p (q d) -> p q d", q=pb - pa),
                        op=ALU.mult)
                    # fold every pair that now has both lanes
                    fb = min(pb, sq_done)
                    if fb > fold_done:
                        fold_pairs(fold_done, fb)
                        fold_done = fb

                    for gi, (ga, gb) in enumerate(fin_groups):
                        ne_hi = gb + 1   # needs NE thru gb (NE16 if gb==Q)
                        ok = fold_done >= min(ne_hi, Q) and (
                            ne_hi <= Q or hi == Q + 1)
                        if gi not in fired and ok and fold_done >= gb:
                            fired.add(gi)
                            _emit_finals(nc, stat, scr, mybir, SN, NE16, OUTB,
                                         ga, gb, gi)

                if use_kv:
                    nc.gpsimd.trigger_dma(count=None)
                else:
                    nc.sync.dma_start(out=out[:, 0:Q], in_=OUTB[:, 0:Q])

    nc.compile()
    _fix_prep_sems(nc, mybir)
    return nc


def _fix_prep_sems(nc, mybir):
    """Point each SWDGE prep's baked DMA-completion sem at its Tile lane sem.

    Tile assigns every Pool DMA inst (including gen_mode==1 preps) a DMASW
    lane and makes downstream waiters wait on that lane's semaphore, but the
    prepare_only API bakes the caller-provided sem into the descriptor
    (on_update[0]) and Tile never rewrites it — so the lane sem would never
    fire.  Rewrite on_update[0] to the lane sem the waiters expect.
    """
    from concourse.tile_sem_assignment import PROC_NAME_TO_IDX

    idx_to_lane = {v: k for k, v in PROC_NAME_TO_IDX.items()}
    by_lane = {}
    for sid, names in nc.m.ant_sem_names.items():
        for nm in names:
            by_lane.setdefault(nm.split("_")[0], (int(sid), nm))
    for f in nc.m.functions:
        for blk in f.blocks:
            for inst in blk.instructions:
                if getattr(inst, "gen_mode", 0) != 1:
                    continue
                lane = idx_to_lane.get(inst.bass_scheduled_proc)
                if lane is None or lane not in by_lane:
                    continue
                sid, nm = by_lane[lane]
                u0 = inst.sync_info.on_update[0]
                inst.sync_info.on_update[0] = mybir.SyncUpdate(
                    sync_type=u0.sync_type, id=sid, ant_name=nm,
                    update_mode=u0.update_mode, update_value=16)


def _emit_finals(nc, stat, scr, mybir, SN, NE16, OUTB, ga, gb, gi):
    """OUTB[:, ga:gb] = (S + c*sqrt(m))^2 / m for block range [ga, gb)."""
    ALU = mybir.AluOpType
    f32 = mybir.dt.float32
    AF = mybir.ActivationFunctionType
    w_ = gb - ga
    SNt = SN.rearrange("p (t s) -> p t s", s=2)
    Sv = SNt[:, ga:gb, 0:1]
    NEv = SNt[:, :, 1:2]
    m = stat.tile([P, w_], f32, name=f"m{gi}")
    m3 = m.rearrange("p (t s) -> p t s", s=1)
    if gb == Q:   # last block's m needs the halo norm NE16
        nc.vector.tensor_tensor(out=m3[:, 0:w_ - 1], in0=NEv[:, ga:gb - 1],
                                in1=NEv[:, ga + 1:gb], op=ALU.mult)
        nc.vector.tensor_tensor(out=m[:, w_ - 1:w_], in0=SNt[:, gb - 1, 1:2],
                                in1=NE16, op=ALU.mult)
    else:
        nc.vector.tensor_tensor(out=m3, in0=NEv[:, ga:gb],
                                in1=NEv[:, ga + 1:gb + 1], op=ALU.mult)
    w = stat.tile([P, w_], f32, name=f"w{gi}")
    nc.vector.reciprocal(w, m)   # off the sqrt chain; joins at the end
    s0 = stat.tile([P, w_], f32, name=f"s0{gi}")
    nc.scalar.activation(out=s0, in_=m, func=AF.Sqrt)
    u = stat.tile([P, w_], f32, name=f"u{gi}")
    nc.vector.scalar_tensor_tensor(out=u.rearrange("p (t s) -> p t s", s=1),
                                   in0=s0.rearrange("p (t s) -> p t s", s=1),
                                   scalar=C_CONST, in1=Sv,
                                   op0=ALU.mult, op1=ALU.add)
    v = stat.tile([P, w_], f32, name=f"v{gi}")
    nc.vector.tensor_tensor(out=v, in0=u, in1=u, op=ALU.mult)
    nc.vector.tensor_tensor(out=OUTB[:, ga:gb], in0=v, in1=w, op=ALU.mult)


def _get_nc():
    if "nc" not in _CACHE:
        _CACHE["nc"] = _build_nc()
    return _CACHE["nc"]


def make_in_maps(x: np.ndarray) -> list[dict[str, np.ndarray]]:
    """Host-side sharding: build each core's SBUF image xh[128, 2176] fp16."""
    x = np.asarray(x, dtype=np.float32).astype(np.float16)
    # pad one row (the out-of-range halo of the last core) with ones
    xp = np.concatenate([x, np.ones((1, D), dtype=np.float16)], axis=0)
    in_maps = []
    for c in range(N_CORES):
        sh = xp[c * R:c * R + R].reshape(P, Q * D)        # blocks 0..15
        halo = xp[c * R + 16 * np.arange(1, P + 1)]       # block 16
        xh = np.concatenate([sh, halo.reshape(P, D)], axis=1)
        in_maps.append({"xh": np.ascontiguousarray(xh)})
    return in_maps


def kernel(feature_clusters: np.ndarray) -> np.ndarray:
    from concourse.bass_utils import run_bass_kernel_spmd

    nc = _get_nc()
    in_maps = make_in_maps(feature_clusters)
    res = run_bass_kernel_spmd(nc, in_maps, list(range(N_CORES))).results
    full = np.concatenate(
        [res[c]["out"][:, :Q].reshape(R) for c in range(N_CORES)])
    return full[:N_ROWS - 1].astype(np.float32)


# revision 22
# speedup vs baseline: 1.6941x; 1.0402x over previous
"""CollapseLoss kernel for Trainium2, 8-way row-sharded, fp16 datapath.

Reference computation (N=16384 rows, D=128):
    x_n   = row-normalize(feature_clusters)            # F.normalize(dim=1)
    d[i]  = dot(x_n[i+1], x_n[i])        i = 0..N-2
    out   = (d + 1/(N-1))**2

Sharding: 2048 rows per core.  Host-side sharding materializes, per core, the
SBUF image xh[128, 17*128] IN FP16: partition p holds rows 16p..16p+15 of the
shard (blocks 0..15) followed by row 16(p+1) (block 16 — the t=15 partner
row; for p=127 it is the next shard's first row: the halo).  fp16 halves the
HBM roofline and unlocks the DVE 2x_1p perf mode.

Raw (unnormalized) dots S and squared norms NE are computed in fp16; the
normalization happens algebraically in the f32 finals:
    out = (S + c*sqrt(m))^2 / m,   m = NE[t] * NE[t+1]
(sqrt only scales the tiny c term, so the loose ACT Sqrt table is ample).

Structure / engine split:
  - First input chunk is a SWDGE dma_gather prepared at t~0 on GPSIMD and
    fired by trigger_dma: the transfer starts ~800ns before the HWDGE path
    could.  Remaining chunks ride HWDGE (desc-gen pipelines under chunk 1's
    transfer).  Identity gather indices come from a GPSIMD iota.
  - Products (DVE TT) and squares (ACT Square) write an interleaved
    PRSQ[p, t, {prod,sq}, 128] buffer so ONE halving fold chain
    (TT adds at fp16 2x) + one short TensorReduce yields the interleaved
    [S, NE] per-block sums — TensorReduce has no fp16 fast path, so folding
    128->16 first is ~2x cheaper than a straight reduce.
  - Block 16's norm (the halo) goes through an ACT Square+accum.
  - The output store is a SWDGE dma_scatter_add prepared early and fired by
    trigger_dma after the finals: tail cost is ~60ns dispatch + transfer
    + sem instead of the HWDGE's 625+650 desc-gen/DGE latency.  The DRAM
    out buffer ([128, 64] padded rows, host slices [:, :16]) is zeroed by an
    early inline GPSIMD store of the memset OUTB tile (scatter ADDs).
"""

import sys
import numpy as np
from contextlib import ExitStack

try:
    import concourse  # noqa: F401
except ImportError:  # grading env without the sitecustomize path
    for _p in ("/opt/trn_rl_repo", "/root/.axon_site/_ro/trn_rl_repo"):
        if _p not in sys.path:
            sys.path.append(_p)

N_ROWS = 16384
D = 128
N_CORES = 8
R = N_ROWS // N_CORES  # 2048 rows per core
P = 128                # partitions
Q = R // P             # 16 row-blocks per partition
C_CONST = 1.0 / (N_ROWS - 1)
OUTW = 64              # padded out row width (scatter elem must be 256B)

CFG = {
    # (lo, hi, mode): input chunks in block units; block 16 = halo blk.
    # NOTE: "gather" (SWDGE dma_gather prep+trigger) starts the first
    # transfer ~800ns earlier in the model but produced flaky data/crashes
    # on the real axon cores, so loads stay on the HWDGE path.
    "chunks": ((0, 6, "hwdge"), (6, 12, "hwdge"), (12, 17, "hwdge")),
    "finals_groups": ((0, 16),),
    "fold_to": 16,
    "store_mode": "kv",   # kv | hwdge
    # sqrt(m) handling for the c*sqrt(m) term: "amgm" approximates sqrt(m)
    # by (NE_t + NE_{t+1})/2 (AM-GM, <=3% err on a term that is ~1.5% of the
    # output), shortening the finals chain; "magic" is the bitcast sqrt;
    # "act" the ACT Sqrt table.
    "finals_mode": "amgm",
    "repeat": 1,
}

SQRT_MAGIC = 0x1FBD1DF5  # bitcast(i>>1 + magic) ~= sqrt, rel err <= 4.5%

_CACHE = {}


def _plan(chunks):
    """Product ranges and paired-fold ranges implied by the chunk bounds.

    Products for blocks [a,b) read AB blocks a..b; a fold pair t needs both
    the product t and the square t (squares land with their chunk).
    """
    prod, pstart = [], 0
    for i, (lo, hi, _m) in enumerate(chunks):
        last = i == len(chunks) - 1
        pend = Q if last else min(hi - 1, Q)
        if pend > pstart:
            prod.append((pstart, pend))
            pstart = pend
    return prod


def _build_nc(cfg=None):
    import concourse.bacc as bacc
    import concourse.tile as tile
    from concourse import mybir, library_config

    cfg = dict(CFG, **(cfg or {}))
    f32 = mybir.dt.float32
    f16 = mybir.dt.float16
    i16 = mybir.dt.int16
    i32 = mybir.dt.int32
    AF = mybir.ActivationFunctionType
    ALU = mybir.AluOpType
    X = mybir.AxisListType.X
    fold_to = cfg["fold_to"]
    chunks = cfg["chunks"]
    use_kv = cfg["store_mode"] == "kv"
    any_gather = any(m == "gather" for _, _, m in chunks)

    nc = bacc.Bacc(
        "TRN2",
        target_bir_lowering=False,
        debug=False,
        enable_asserts=False,
        num_devices=N_CORES,
    )
    xh = nc.dram_tensor("xh", [P, (Q + 1) * D], f16, kind="ExternalInput").ap()
    out = nc.dram_tensor("out", [P, OUTW], f32, kind="ExternalOutput").ap()

    prod_ranges = _plan(chunks)

    with tile.TileContext(nc) as tc:
        with ExitStack() as ctx:
            data = ctx.enter_context(tc.tile_pool(name="data", bufs=1))
            scr = ctx.enter_context(tc.tile_pool(name="scr", bufs=2))
            stat = ctx.enter_context(tc.tile_pool(name="stat", bufs=1))

            for _rep in range(cfg["repeat"]):
                # PRSQ slot 2t = prod_t (t<16), slot 2t+1 = sq_t (t<=16, so
                # slot 33 = halo square); slot 32 is never written (memset
                # once) and folds into the unused SN[:, 32].
                NS = 2 * Q + 2   # 34 slots
                AB = data.tile([P, (Q + 1) * D], f16, name=f"AB{_rep}")
                PRSQ = data.tile([P, NS * D], f16)
                F1 = data.tile([P, NS * 64], f16)
                F2 = data.tile([P, NS * 32], f16)
                F3 = data.tile([P, NS * 16], f16)
                SN = stat.tile([P, NS], f32)            # interleaved S/NE
                OUTB = stat.tile([P, OUTW], f32)

                # ---- GPSIMD stream: library, identity idxs, preps/triggers
                if any_gather or use_kv:
                    nc.gpsimd.load_library(library_config.attnmlp)
                if any_gather:
                    idxs = scr.tile([16, 8], i16, name=f"idx{_rep}")
                    # slot i of the SWDGE ring reads idxs[i%16, i//16]; we
                    # want slot i -> row i (identity).
                    nc.gpsimd.iota(idxs, pattern=[[16, 8]], base=0,
                                   channel_multiplier=1)

                # hoist the single ACT table load (Sqrt set) to t~0
                dum = scr.tile([P, 1], f32, name=f"dum{_rep}")
                one = nc.const_aps.aps[(f32, 1.0)]
                nc.scalar.activation(out=dum, in_=one[:P], func=AF.Sqrt)

                # ---- input loads
                for ci, (lo, hi, mode) in enumerate(chunks):
                    w = (hi - lo) * D
                    if mode == "gather":
                        sem = nc.alloc_semaphore(f"gat{_rep}_{ci}")
                        dst = AB[:, lo * D:hi * D].rearrange(
                            "p (s e) -> p s e", s=1)
                        nc.gpsimd.dma_gather(
                            dst, xh[:, lo * D:hi * D], idxs, P, P, w,
                            elem_step=(Q + 1) * D,
                            prepare_only=True, sem=sem)
                        nc.gpsimd.trigger_dma(count=None)
                    else:
                        nc.sync.dma_start(out=AB[:, lo * D:hi * D],
                                          in_=xh[:, lo * D:hi * D])



                # ---- store prep (early; OUTB read deferred to the trigger)
                if use_kv:
                    ctxz = scr.tile([P, 1], i32, name=f"ctx{_rep}")
                    nc.gpsimd.memset(ctxz, 0)
                    ssem = nc.alloc_semaphore(f"kv{_rep}")
                    out4 = out.rearrange("p (b o q) -> b p o q", b=1, o=1)
                    in4 = OUTB[:, 0:Q].rearrange("p (o b q) -> p o b q",
                                                 o=1, b=1)
                    nc.gpsimd.kv_writeback(out4, in4, ctxz,
                                           prepare_only=True, sem=ssem)

                AB3 = AB.rearrange("p (q d) -> p q d", q=Q + 1)
                PRSQt = PRSQ.rearrange("p (t x) -> p t x", t=Q + 1)  # x=2*128
                PRSQu = PRSQ.rearrange("p (u d) -> p u d", u=NS)
                V1 = F1.rearrange("p (u d) -> p u d", u=NS)
                V2 = F2.rearrange("p (u d) -> p u d", u=NS)
                V3 = F3.rearrange("p (u d) -> p u d", u=NS)

                def fold_pairs(fa, fb):
                    """Fold PRSQ pair-blocks [fa,fb) down to SN[:, 2fa:2fb]."""
                    ua, ub = 2 * fa, 2 * fb
                    nc.vector.tensor_tensor(
                        out=V1[:, ua:ub], in0=PRSQu[:, ua:ub, 0:64],
                        in1=PRSQu[:, ua:ub, 64:128], op=ALU.add)
                    cur, width = V1, 64
                    for nxt, nw in ((V2, 32), (V3, 16)):
                        if width <= fold_to:
                            break
                        nc.vector.tensor_tensor(
                            out=nxt[:, ua:ub], in0=cur[:, ua:ub, 0:nw],
                            in1=cur[:, ua:ub, nw:2 * nw], op=ALU.add)
                        cur, width = nxt, nw
                    nc.vector.tensor_reduce(SN[:, ua:ub], cur[:, ua:ub],
                                            axis=X, op=ALU.add)

                fin_groups = cfg["finals_groups"]
                fired = set()
                fold_done = 0
                sq_done = 0
                for ri, (pa, pb) in enumerate(prod_ranges):
                    lo, hi, _m = chunks[ri]
                    # squares on ACT; the last range covers the halo block
                    # whose square lands in slot 33
                    qa, qb = sq_done, min(hi, Q + 1)
                    if qb > qa:
                        nc.scalar.activation(
                            out=PRSQt[:, qa:qb, D:2 * D],
                            in_=AB3[:, qa:qb, :], func=AF.Square)
                        sq_done = qb
                    if qb == Q + 1:
                        # fill the dead prod slot 32 with the halo square too
                        # (fold lane must be finite; SN[:, 32] is unused)
                        nc.scalar.activation(
                            out=PRSQ[:, 32 * D:33 * D],
                            in_=AB3[:, Q, :], func=AF.Square)
                    # shifted products on DVE
                    nc.vector.tensor_tensor(
                        out=PRSQt[:, pa:pb, 0:D],
                        in0=AB3[:, pa:pb, :],
                        in1=AB[:, pa * D + D:pb * D + D].rearrange(
                            "p (q d) -> p q d", q=pb - pa),
                        op=ALU.mult)
                    # fold every pair with both lanes ready (pair 16 has
                    # only the sq lane; its prod slot 32 is the memset slot)
                    fb = Q + 1 if sq_done == Q + 1 else min(pb, sq_done)
                    if fb > fold_done:
                        fold_pairs(fold_done, fb)
                        fold_done = fb

                    for gi, (ga, gb) in enumerate(fin_groups):
                        if gi not in fired and fold_done >= gb + 1:
                            fired.add(gi)
                            _emit_finals(nc, stat, mybir, SN, OUTB,
                                         ga, gb, gi, cfg)

                if use_kv:
                    nc.gpsimd.trigger_dma(count=None)
                else:
                    nc.sync.dma_start(out=out[:, 0:Q], in_=OUTB[:, 0:Q])

    nc.compile()
    _fix_prep_sems(nc, mybir)
    return nc


def _fix_prep_sems(nc, mybir):
    """Point each SWDGE prep's baked DMA-completion sem at its Tile lane sem.

    Tile assigns every Pool DMA inst (including gen_mode==1 preps) a DMASW
    lane and makes downstream waiters wait on that lane's semaphore, but the
    prepare_only API bakes the caller-provided sem into the descriptor
    (on_update[0]) and Tile never rewrites it — so the lane sem would never
    fire.  Rewrite on_update[0] to the lane sem the waiters expect.
    """
    from concourse.tile_sem_assignment import PROC_NAME_TO_IDX

    idx_to_lane = {v: k for k, v in PROC_NAME_TO_IDX.items()}
    by_lane = {}
    for sid, names in nc.m.ant_sem_names.items():
        for nm in names:
            by_lane.setdefault(nm.split("_")[0], (int(sid), nm))
    for f in nc.m.functions:
        for blk in f.blocks:
            for inst in blk.instructions:
                if getattr(inst, "gen_mode", 0) != 1:
                    continue
                lane = idx_to_lane.get(inst.bass_scheduled_proc)
                if lane is None or lane not in by_lane:
                    continue
                sid, nm = by_lane[lane]
                u0 = inst.sync_info.on_update[0]
                inst.sync_info.on_update[0] = mybir.SyncUpdate(
                    sync_type=u0.sync_type, id=sid, ant_name=nm,
                    update_mode=u0.update_mode, update_value=16)


def _emit_finals(nc, stat, mybir, SN, OUTB, ga, gb, gi, cfg):
    """OUTB[:, ga:gb] = (S + c*sqrt(m))^2 / m for block range [ga, gb)."""
    ALU = mybir.AluOpType
    f32 = mybir.dt.float32
    i32 = mybir.dt.int32
    AF = mybir.ActivationFunctionType
    w_ = gb - ga
    SNt = SN.rearrange("p (t s) -> p t s", s=2)
    Sv = SNt[:, ga:gb, 0:1]
    NEv = SNt[:, :, 1:2]
    m = stat.tile([P, w_], f32, name=f"m{gi}")
    m3 = m.rearrange("p (t s) -> p t s", s=1)
    nc.vector.tensor_tensor(out=m3, in0=NEv[:, ga:gb],
                            in1=NEv[:, ga + 1:gb + 1], op=ALU.mult)
    w = stat.tile([P, w_], f32, name=f"w{gi}")
    nc.vector.reciprocal(w, m)   # off the sqrt chain; joins at the end
    u = stat.tile([P, w_], f32, name=f"u{gi}")
    u3 = u.rearrange("p (t s) -> p t s", s=1)
    mode = cfg["finals_mode"]
    if mode == "amgm":
        # sqrt(NE_t*NE_t1) ~= (NE_t+NE_t1)/2; u = S + (c/2)*(NE_t+NE_t1)
        ns_ = stat.tile([P, w_], f32, name=f"ns{gi}")
        ns3 = ns_.rearrange("p (t s) -> p t s", s=1)
        nc.vector.tensor_tensor(out=ns3, in0=NEv[:, ga:gb],
                                in1=NEv[:, ga + 1:gb + 1], op=ALU.add)
        nc.vector.scalar_tensor_tensor(out=u3, in0=ns3, scalar=C_CONST / 2,
                                       in1=Sv, op0=ALU.mult, op1=ALU.add)
    else:
        s0 = stat.tile([P, w_], f32, name=f"s0{gi}")
        if mode == "act":
            nc.scalar.activation(out=s0, in_=m, func=AF.Sqrt)
        else:
            sh = stat.tile([P, w_], f32, name=f"sh{gi}")
            nc.vector.tensor_scalar(sh.bitcast(i32), m.bitcast(i32), 1, None,
                                    ALU.logical_shift_right)
            nc.vector.tensor_scalar(s0.bitcast(i32), sh.bitcast(i32),
                                    SQRT_MAGIC, None, ALU.add)
        nc.vector.scalar_tensor_tensor(out=u3,
                                       in0=s0.rearrange("p (t s) -> p t s",
                                                        s=1),
                                       scalar=C_CONST, in1=Sv,
                                       op0=ALU.mult, op1=ALU.add)
    v = stat.tile([P, w_], f32, name=f"v{gi}")
    nc.vector.tensor_tensor(out=v, in0=u, in1=u, op=ALU.mult)
    nc.vector.tensor_tensor(out=OUTB[:, ga:gb], in0=v, in1=w, op=ALU.mult)


def _get_nc():
    if "nc" not in _CACHE:
        _CACHE["nc"] = _build_nc()
    return _CACHE["nc"]


def make_in_maps(x: np.ndarray) -> list[dict[str, np.ndarray]]:
    """Host-side sharding: build each core's SBUF image xh[128, 2176] fp16."""
    x = np.asarray(x, dtype=np.float32).astype(np.float16)
    # pad one row (the out-of-range halo of the last core) with ones
    xp = np.concatenate([x, np.ones((1, D), dtype=np.float16)], axis=0)
    in_maps = []
    for c in range(N_CORES):
        sh = xp[c * R:c * R + R].reshape(P, Q * D)        # blocks 0..15
        halo = xp[c * R + 16 * np.arange(1, P + 1)]       # block 16
        xh = np.concatenate([sh, halo.reshape(P, D)], axis=1)
        in_maps.append({"xh": np.ascontiguousarray(xh)})
    return in_maps


def kernel(feature_clusters: np.ndarray) -> np.ndarray:
    from concourse.bass_utils import run_bass_kernel_spmd

    nc = _get_nc()
    in_maps = make_in_maps(feature_clusters)
    res = run_bass_kernel_spmd(nc, in_maps, list(range(N_CORES))).results
    full = np.concatenate(
        [res[c]["out"][:, :Q].reshape(R) for c in range(N_CORES)])
    return full[:N_ROWS - 1].astype(np.float32)
